# revision 77
# baseline (speedup 1.0000x reference)
"""Trainium2 Bass kernel for nn_Attention (B=2, N=4096, DIM=768, H=12 heads).

Sharding: 24 (batch, head) pairs over 8 cores -> 3 heads per core, 4 cores
per batch element. Each core computes, for its batch b and its 3 heads:
  q,k,v projections -> full attention (flash-style, no score materialization
  to HBM) -> partial output projection  y_partial^T = sum_h wp_h^T @ O_h^T.
The host sums the 4 partials per batch and adds the bias.

Device dataflow (all matmul inputs bf16, fp32 PSUM accumulation):
  - host passes x^T, w_qk^T, w_v^T, w_p^T pre-transposed/pre-sliced in bf16
  - qk^T = [w_q|w_k]^T.T @ x^T    -> q^T,k^T [64, 4096] per head (d-major)
  - S^T[k,q] = k^T.T @ q^T        -> PSUM, 2-way row-packed (K=64)
  - P^T = exp(S^T * scale)        -> ScalarE (the bottleneck: 50.3M exps
    per core at 1 elem/cycle/lane @ 1.2 GHz = 327.7us hard floor)
  - O^T|den = [V|1].T @ P^T       -> PSUM accumulate over k blocks
  - O^T /= den (recip + gpsimd partition-broadcast + DVE mult; recip
    source MUST be a partition-0 SBUF tile — anything else silently
    misreads on HW)
  - y^T += wp_h^T.T @ O_h^T       -> per-head K=64 accumulation

The schedule is a software pipeline: each "cycle" emits tile t's 16 S
groups interleaved ~1:1 with tile t-1's AV groups, so the 3-slot PSUM
score ring refills continuously and ScalarE never drains its (ring-
capped) backlog — zero steady-state exp gaps. qk chains and output-
projection chains ride in per-cycle filler slots spaced two S groups
apart so their PSUM-bank write-after-read latency hides behind the
intervening S pairs.
"""

import numpy as np
import ml_dtypes

import concourse.bacc as bacc
import concourse.mybir as mybir
import concourse.tile as tile
from concourse.bass_utils import run_bass_kernel_spmd

BF16 = mybir.dt.bfloat16
F32 = mybir.dt.float32

DIM = 768
N = 4096
NUM_HEADS = 12
HEAD_DIM = 64
SCALE = HEAD_DIM ** -0.5
B = 2
NCORES = 8
HPC = 3  # heads per core
CCH = DIM // 128  # 6 contraction chunks of 128
NQT = 8  # q tiles of 512
QT = 512
NKB = 32  # k blocks of 128
KB = 128
# exp groups: 3 k-blocks per ACTIVATE (one [128,1536] f32 = 3 PSUM banks)
# amortizes the ~260-cycle per-instruction ScalarE overhead over 50% more
# elements; the ragged last group covers the remaining 2 k-blocks.
GROUPS = [(3 * i, 3) for i in range(10)] + [(30, 2)]


def build_program():
    nc = bacc.Bacc("TRN2", target_bir_lowering=False, debug=False)

    xT = nc.dram_tensor("xT", [DIM, N], BF16, kind="ExternalInput")
    wqkT = nc.dram_tensor("wqkT", [DIM, HPC * 128], BF16, kind="ExternalInput")
    wvT = nc.dram_tensor("wvT", [DIM, HPC * 64], BF16, kind="ExternalInput")
    wpT = nc.dram_tensor("wpT", [HPC * 64, DIM], BF16, kind="ExternalInput")
    yT = nc.dram_tensor("yT", [DIM, N], F32, kind="ExternalOutput")

    ngrp = len(GROUPS)

    with tile.TileContext(nc) as tc:
        with (
            tc.tile_pool(name="wpool", bufs=1) as wpool,
            tc.tile_pool(name="qkpool", bufs=1) as qkpool,
            tc.tile_pool(name="pspool", bufs=2, space="PSUM") as pspool,
            tc.tile_pool(name="vpool", bufs=1, space="PSUM") as vpool,
            tc.tile_pool(name="accpool", bufs=1, space="PSUM") as accpool,
            tc.tile_pool(name="espool", bufs=16) as espool,
            tc.tile_pool(name="dpool", bufs=2) as dpool,
            tc.tile_pool(name="opool", bufs=19) as opool,
            tc.tile_pool(name="ypool", bufs=3) as ypool,
        ):
            # xT split into one tile per q-token-tile so phase A can start as
            # soon as the first slice lands (per-tile DMA dependencies).
            xTs = [
                wpool.tile([128, CCH * QT], BF16, tag=f"xT{qt}", name=f"xT{qt}")
                for qt in range(NQT)
            ]
            wqk_sb = wpool.tile([128, CCH * HPC * 128], BF16, tag="wqk")
            wv_sb = wpool.tile([128, CCH * HPC * 64], BF16, tag="wv")
            wp_sb = wpool.tile([64, HPC * DIM], BF16, tag="wp")
            T = [
                qkpool.tile([128, 2 * N], BF16, tag=f"T{h}", name=f"T{h}")
                for h in range(HPC)
            ]
            V = [
                qkpool.tile([128, NKB * 65], BF16, tag=f"V{h}", name=f"V{h}")
                for h in range(HPC)
            ]

            # HAM warm-up: the PE clock sits at 1.2 GHz until ~3.4us of
            # sustained matmul activity. Zero-tile dummy matmuls during the
            # initial DMA wait flip it to 2.4 GHz before the first real qk
            # chain, and keep it from re-throttling before work arrives.
            # The dummy exp also hoists the ACT table load (~2.7us) into the
            # DMA shadow.
            warm = dpool.tile([128, QT], BF16, tag="warm", bufs=1)
            nc.gpsimd.memset(warm[:], 0.0)
            wes = dpool.tile([1, 8], BF16, tag="wes", bufs=1)
            nc.scalar.activation(
                wes[:], warm[0:1, 0:8], mybir.ActivationFunctionType.Exp,
                scale=1.0,
            )

            def emit_warm_mms(n, name):
                for w in range(n):
                    wps = pspool.tile([128, QT], F32, tag="s",
                                      name=f"{name}{w}")
                    nc.tensor.matmul(
                        wps[:], lhsT=warm[:, 0:128], rhs=warm[:, 0:QT],
                        start=True, stop=True,
                    )

            emit_warm_mms(16, "wps")

            # DMA critical path: the first qk matmul needs only (wqk c0,
            # xT0 c0) — issue those two first, then the remainders as one
            # consolidated transfer each (4 Sync issues instead of 12; the
            # big transfers complete well before the chain reaches chunk 1).
            wqk_src = wqkT[:].rearrange("(c p) n -> p c n", p=128)
            xT_src = xT[:].rearrange("(c p) n -> p c n", p=128)
            nc.sync.dma_start(out=wqk_sb[:, 0:384], in_=wqkT[0:128, :])
            nc.sync.dma_start(out=xTs[0][:, 0:QT], in_=xT[0:128, 0:QT])
            nc.sync.dma_start(
                out=wqk_sb[:, 384:].rearrange("p (c n) -> p c n", n=384),
                in_=wqk_src[:, 1:, :],
            )
            nc.sync.dma_start(
                out=xTs[0][:, QT:].rearrange("p (c n) -> p c n", n=QT),
                in_=xT_src[:, 1:, 0:QT],
            )
            xT_loaded = {0}

            def load_xT(qt, chunked=False):
                if qt in xT_loaded:
                    return
                xT_loaded.add(qt)
                if chunked:
                    for c in range(CCH):
                        nc.sync.dma_start(
                            out=xTs[qt][:, c * QT:(c + 1) * QT],
                            in_=xT[c * 128:(c + 1) * 128, qt * QT:(qt + 1) * QT],
                        )
                    return
                nc.sync.dma_start(
                    out=xTs[qt][:].rearrange("p (c n) -> p c n", n=QT),
                    in_=xT_src[:, :, qt * QT:(qt + 1) * QT],
                )
            def load_w2():
                nc.sync.dma_start(
                    out=wv_sb[:].rearrange("p (c n) -> p c n", n=192),
                    in_=wvT[:].rearrange("(c p) n -> p c n", p=128),
                )
                nc.sync.dma_start(
                    out=wp_sb[0:64, :].rearrange("p (h n) -> p h n", n=DIM),
                    in_=wpT[:].rearrange("(h p) n -> p h n", p=64),
                )
            for h in range(HPC):
                nc.gpsimd.memset(V[h][:], 1.0)

            # ---- emission helpers ----
            def emit_qk_tile(h, qt, chunked=False):
                load_xT(qt, chunked=chunked)
                ps = pspool.tile([128, QT], F32, tag="s", name=f"qk{h}_{qt}")
                for c in range(CCH):
                    nc.tensor.matmul(
                        ps[:],
                        lhsT=wqk_sb[:, c * 384 + h * 128: c * 384 + (h + 1) * 128],
                        rhs=xTs[qt][:, c * QT:(c + 1) * QT],
                        start=(c == 0),
                        stop=(c == CCH - 1),
                    )
                nc.vector.tensor_copy(
                    T[h][0:64, N + qt * QT: N + (qt + 1) * QT], ps[0:64, :]
                )
                nc.vector.tensor_copy(
                    T[h][64:128, qt * QT:(qt + 1) * QT], ps[64:128, :]
                )

            def emit_qk_dup(h, qt=None, dve=False):
                if qt is None:
                    nc.sync.dma_start(out=T[h][0:64, 0:N], in_=T[h][64:128, 0:N])
                    nc.sync.dma_start(
                        out=T[h][64:128, N:2 * N], in_=T[h][0:64, N:2 * N]
                    )
                elif dve:
                    # ramp path: DVE bf16 copies (~200ns each) beat the DMA
                    # round-trip latency when the Sync queue is busy issuing
                    # the initial loads.
                    s = qt * QT
                    nc.vector.tensor_copy(
                        T[h][0:64, s:s + QT], T[h][64:128, s:s + QT]
                    )
                    nc.vector.tensor_copy(
                        T[h][64:128, N + s: N + s + QT],
                        T[h][0:64, N + s: N + s + QT],
                    )
                else:
                    s = qt * QT
                    nc.sync.dma_start(
                        out=T[h][0:64, s:s + QT], in_=T[h][64:128, s:s + QT]
                    )
                    nc.sync.dma_start(
                        out=T[h][64:128, N + s: N + s + QT],
                        in_=T[h][0:64, N + s: N + s + QT],
                    )

            def emit_v_tile(tt):
                # ping-pong the v projections across the v bank and the (not
                # yet used) po bank so tile t+1's matmuls don't wait on tile
                # t's copy-out.
                if tt % 2 == 0:
                    ps = vpool.tile([128, HPC * 64], F32, tag="v", name=f"v{tt}")
                else:
                    ps = accpool.tile([128, HPC * 64], F32, tag="o", name=f"v{tt}")
                for c in range(CCH):
                    nc.tensor.matmul(
                        ps[:],
                        lhsT=xTs[tt // 4][:, c * QT + (tt % 4) * 128: c * QT + (tt % 4) * 128 + 128],
                        rhs=wv_sb[:, c * 192:(c + 1) * 192],
                        start=(c == 0),
                        stop=(c == CCH - 1),
                    )
                for h in range(HPC):
                    nc.vector.tensor_copy(
                        V[h][:, tt * 65: tt * 65 + 64],
                        ps[:, h * 64:(h + 1) * 64],
                    )

            es_store = {}

            def emit_s_group(h, qt, g):
                kb0, nkb = GROUPS[g]
                ps = pspool.tile([128, nkb * QT], F32, tag="s",
                                 padded_shape=[128, 3 * QT],
                                 name=f"ps{h}_{qt}_{g}")
                es = espool.tile([128, nkb * QT], BF16, tag="es",
                                 padded_shape=[128, 3 * QT],
                                 name=f"es{h}_{qt}_{g}")
                for j in range(nkb):
                    kb = kb0 + j
                    o = 64 * (kb % 2)
                    nc.tensor.matmul(
                        ps[:, j * QT:(j + 1) * QT],
                        lhsT=T[h][o:o + 64, kb * KB:(kb + 1) * KB],
                        rhs=T[h][o:o + 64, N + qt * QT: N + (qt + 1) * QT],
                        start=True,
                        stop=True,
                    )
                nc.scalar.activation(
                    es[:], ps[:], mybir.ActivationFunctionType.Exp, scale=SCALE
                )
                es_store[(h, qt, g)] = es

            def emit_av_group(h, qt, g, po):
                es = es_store.pop((h, qt, g))
                kb0, nkb = GROUPS[g]
                for j in range(nkb):
                    kb = kb0 + j
                    nc.tensor.matmul(
                        po[:],
                        lhsT=V[h][:, kb * 65: kb * 65 + 65],
                        rhs=es[:, j * QT:(j + 1) * QT],
                        start=(kb == 0),
                        stop=(kb == NKB - 1),
                        skip_group_check=True,
                    )

            O = [[None] * HPC for _ in range(NQT)]

            def emit_norm(h, qt, po):
                # two early PSUM->SBUF copies release the po bank for the
                # next tile's AV immediately. reciprocal_approx_fast needs a
                # partition-0 SBUF source — reading it from PSUM or from a
                # partition-64 slice silently misreads on HW (sim-clean).
                sm = dpool.tile([64, QT], BF16, tag="sm", name=f"sm{h}_{qt}")
                nc.vector.tensor_copy(sm[:], po[0:64, :])
                dr0 = dpool.tile([1, QT], F32, tag="dr0", name=f"dr0_{h}_{qt}")
                nc.vector.tensor_copy(dr0[:], po[64:65, :])
                dr = dpool.tile([1, QT], F32, tag="dr", name=f"dr{h}_{qt}")
                nc.vector.reciprocal_approx_fast(out=dr[:], in_=dr0[:])
                db = dpool.tile([64, QT], F32, tag="db", name=f"db{h}_{qt}")
                nc.gpsimd.partition_broadcast(db[:], dr[:])
                oh = opool.tile([64, QT], BF16, tag="O", name=f"O{h}_{qt}")
                nc.vector.tensor_mul(oh[:], sm[0:64, :], db[:])
                O[qt][h] = oh

            def emit_cycle(cur, prev, fillers=(), start_g=0, win=2):
                """Software-pipeline step: S groups [start_g..ngrp) of tile
                `cur` interleaved with the AV groups (and norm) of tile
                `prev`. Spreading the AV work between S groups keeps the
                3-slot S ring continuously replenished, so ACT never drains
                its (ring-capped) backlog during a monolithic AV block.

                HW CONSTRAINT: only ONE multi-matmul accumulation group may
                be open at a time (interleaving two corrupts PSUM state on
                HW even across different banks — sim-clean, deterministic
                garbage on device). The S matmuls are single-MM groups, so
                they interleave freely with prev's open AV group; multi-MM
                fillers (qk chains, proj chains) are restricted to the first
                `win` S slots, BEFORE the first AV opens prev's group."""
                fmap = {}
                for g, fn in fillers:
                    fmap.setdefault(g, []).append(fn)
                po = None
                if prev is not None:
                    po = accpool.tile(
                        [65, QT], F32, tag="o", name=f"po{prev[0]}_{prev[1]}"
                    )
                av_g = 0
                n_s = ngrp - start_g
                for i, g in enumerate(range(start_g, ngrp)):
                    emit_s_group(cur[0], cur[1], g)
                    for fn in fmap.get(g, ()):
                        assert i < win or prev is None
                        fn()
                    if prev is not None and i >= win:
                        target = (i - win + 1) * ngrp // (n_s - win + 1)
                        while av_g < target:
                            emit_av_group(prev[0], prev[1], av_g, po)
                            av_g += 1
                if prev is not None:
                    while av_g < ngrp:
                        emit_av_group(prev[0], prev[1], av_g, po)
                        av_g += 1
                    emit_norm(prev[0], prev[1], po)

            def emit_proj_chain(qt, oc, pool_tag="v"):
                # share the v-projection bank: the v burst (head-0 phase)
                # and the output projections (head-2 phase) are disjoint.
                if pool_tag == "v":
                    py = vpool.tile([128, QT], F32, tag="v", name=f"py{qt}_{oc}")
                else:
                    py = accpool.tile([128, QT], F32, tag="o", name=f"py{qt}_{oc}")
                for h in range(HPC):
                    nc.tensor.matmul(
                        py[:],
                        lhsT=wp_sb[0:64, h * DIM + oc * 128: h * DIM + (oc + 1) * 128],
                        rhs=O[qt][h][:],
                        start=(h == 0),
                        stop=(h == HPC - 1),
                    )
                ysb = ypool.tile([128, QT], F32, tag="y", name=f"y{qt}_{oc}")
                nc.vector.tensor_copy(ysb[:], py[:])
                nc.sync.dma_start(
                    out=yT[oc * 128:(oc + 1) * 128, qt * QT:(qt + 1) * QT],
                    in_=ysb[:],
                )

            def emit_proj(qt):
                for oc in range(CCH):
                    emit_proj_chain(qt, oc)

            # ---- static schedule ----
            # head 0 qk tiles emitted incrementally with qt0's S groups: the
            # first exp fires after just ONE qk tile (S group g only needs
            # k-blocks 2g,2g+1 -> qk tile g//2, and the q side of tile 0).
            # two-tile lookahead: the S groups emitted after qk tile j belong
            # to tile j-2, whose parity duplication completed long ago — the
            # in-order PE queue never stalls on the CAST->dup chain, so the
            # ramp runs at PE throughput instead of dependency latency.
            # the first two qk chains interleave chunk-by-chunk (separate
            # accumulation groups — safe on HW): each chain's matmuls fill
            # the other's per-chunk DMA stalls, and all four leading S
            # groups become ready together instead of serially.
            load_xT(1, chunked=True)
            psq = [
                pspool.tile([128, QT], F32, tag="s", name=f"qk0_{qt}i")
                for qt in range(2)
            ]
            for c in range(CCH):
                for qt in range(2):
                    nc.tensor.matmul(
                        psq[qt][:],
                        lhsT=wqk_sb[:, c * 384: c * 384 + 128],
                        rhs=xTs[qt][:, c * QT:(c + 1) * QT],
                        start=(c == 0),
                        stop=(c == CCH - 1),
                        skip_group_check=True,
                    )
            for qt in range(2):
                nc.vector.tensor_copy(
                    T[0][0:64, N + qt * QT: N + (qt + 1) * QT],
                    psq[qt][0:64, :],
                )
                nc.vector.tensor_copy(
                    T[0][64:128, qt * QT:(qt + 1) * QT], psq[qt][64:128, :]
                )
                emit_qk_dup(0, qt, dve=True)
            emit_s_group(0, 0, 0)
            emit_s_group(0, 0, 1)
            load_w2()
            # group G needs qk tile (last kb of G)//4; this map paces the 11
            # ragged groups against tile arrivals.
            ramp_groups = {2: (2, 3), 3: (4,), 4: (5,), 5: (6, 7), 6: (8,),
                           7: (9, 10)}
            for j in range(2, NQT):
                emit_qk_tile(0, j)
                emit_qk_dup(0, j, dve=True)
                for g in ramp_groups[j]:
                    emit_s_group(0, 0, g)
                # two v tiles per ramp step fill the PE stalls left by the
                # DMA-paced qk chains and shorten the v burst below.
                emit_v_tile(2 * (j - 2))
                emit_v_tile(2 * (j - 2) + 1)

            # v burst interleaved with (0,1)'s first S groups so ACT stays
            # fed while the PE churns the 32 v projections. The deferred AV
            # for qt0 is folded into cycle (0,1) by the pipeline below.
            for tt in range(12, NKB):
                emit_v_tile(tt)
                if tt % 4 == 3:
                    emit_s_group(0, 1, (tt - 15) // 4)

            def qk_filler(nexth, qt):
                # one qk tile (plus its parity duplication) per q-tile slot,
                # spread across the head's attention instead of a tail burst.
                def fill():
                    emit_qk_tile(nexth, qt)
                    emit_qk_dup(nexth, qt, dve=True)

                return fill

            def proj_fillers(qt):
                # one output-projection chain per window slot; the S group in
                # between covers the py-bank write-after-read latency. Cycle
                # (2,qt) projects qt-2 — O[qt-1][2] is only normalized at the
                # END of this cycle.
                if qt < 2:
                    return []
                return [
                    (oc, lambda q=qt, c=oc: emit_proj_chain(q - 2, c))
                    for oc in range(CCH)
                ]

            tiles = [(0, qt) for qt in range(1, NQT)]
            tiles += [(1, qt) for qt in range(NQT)]
            tiles += [(2, qt) for qt in range(NQT)]
            for i, (h, qt) in enumerate(tiles):
                prev = tiles[i - 1] if i > 0 else (0, 0)
                start_g = 5 if (h, qt) == (0, 1) else 0
                win = 2
                if (h, qt) == (0, 1):
                    fillers = [(5, qk_filler(1, 0)), (6, qk_filler(1, 1))]
                elif h == 0:
                    fillers = [(1, qk_filler(1, qt))]
                elif h == 1:
                    fillers = [(1, qk_filler(2, qt))]
                else:
                    fillers = proj_fillers(qt)
                    win = 6 if fillers else 2
                emit_cycle((h, qt), prev, fillers=fillers, start_g=start_g,
                           win=win)

            # drain: the last tile's AV block, then norm(2,7) immediately (its
            # DVE/gpsimd chain overlaps proj(6) matmuls, which keeps the PE
            # warm through it), then both remaining projections ping-ponging
            # between the v bank and the po bank so the twelve chains
            # pipeline instead of serializing on one slot.
            po_last = accpool.tile([65, QT], F32, tag="o", name="po_last")
            for g in range(ngrp):
                emit_av_group(2, NQT - 1, g, po_last)
                if g % 2 == 1 and g >= 3:
                    emit_proj_chain(NQT - 2, (g - 3) // 2)
            emit_proj_chain(NQT - 2, 4)
            emit_proj_chain(NQT - 2, 5)
            emit_norm(2, NQT - 1, po_last)
            # keep the PE clock warm through the final norm's DVE/gpsimd
            # latency so the last projection runs at 2.4 GHz.
            emit_warm_mms(14, "twps")
            for oc in range(CCH):
                emit_proj_chain(NQT - 1, oc, pool_tag="v" if oc % 2 == 0 else "o")

    nc.compile()
    return nc


def make_in_maps(x, w_qkv):
    """Build the 8 per-core input maps from the full fp32 inputs."""
    bf = ml_dtypes.bfloat16
    in_maps = []
    for core in range(NCORES):
        b = core // 4
        hs = [(core % 4) * HPC + i for i in range(HPC)]
        xTb = np.ascontiguousarray(np.asarray(x[b]).T).astype(bf)
        wqk = np.empty((DIM, HPC * 128), dtype=bf)
        wv = np.empty((DIM, HPC * 64), dtype=bf)
        for i, h in enumerate(hs):
            wqk[:, i * 128: i * 128 + 64] = w_qkv[h * 64:(h + 1) * 64, :].T
            wqk[:, i * 128 + 64: i * 128 + 128] = w_qkv[DIM + h * 64: DIM + (h + 1) * 64, :].T
            wv[:, i * 64:(i + 1) * 64] = w_qkv[2 * DIM + h * 64: 2 * DIM + (h + 1) * 64, :].T
        in_maps.append({"xT": xTb, "wqkT": wqk, "wvT": wv})
    return in_maps


def make_wp_map(core, w_proj):
    bf = ml_dtypes.bfloat16
    hs = [(core % 4) * HPC + i for i in range(HPC)]
    wp = np.empty((HPC * 64, DIM), dtype=bf)
    for i, h in enumerate(hs):
        wp[i * 64:(i + 1) * 64, :] = w_proj[:, h * 64:(h + 1) * 64].T
    return wp


_NC = None


def kernel(x, w_qkv, w_proj, b_proj):
    global _NC
    if _NC is None:
        _NC = build_program()
    x = np.asarray(x, dtype=np.float32)
    w_qkv = np.asarray(w_qkv, dtype=np.float32)
    w_proj = np.asarray(w_proj, dtype=np.float32)
    b_proj = np.asarray(b_proj, dtype=np.float32)

    in_maps = make_in_maps(x, w_qkv)
    for core in range(NCORES):
        in_maps[core]["wpT"] = make_wp_map(core, w_proj)

    r = run_bass_kernel_spmd(_NC, in_maps, list(range(NCORES)))
    y = np.zeros((B, N, DIM), dtype=np.float32)
    for core in range(NCORES):
        b = core // 4
        y[b] += r.results[core]["yT"].T
    y += b_proj[None, None, :]
    return y

